# revision 1
# baseline (speedup 1.0000x reference)
"""Trainium2 Bass kernel for nn_CLUB_816043786555 (CLUB loss).

Full-input contract: kernel(**inputs) takes the complete arrays, shards the
batch dim across 8 NeuronCores, runs a Bass/Tile kernel per core, and
combines tiny per-core partial sums on the host.

Math: with mu = leaky(x@W1m+b1m)@W2m+b2m, logvar = tanh(leaky(x@W1v+b1v)@W2v+b2v),
iv = exp(-logvar), ym_d = mean_i y, y2m_d = mean_i y^2:

  loss = -0.5/N * sum_{i,d} iv*(y^2 - 2*mu*y - y2m + 2*mu*ym)
       = -0.5/N * [ P1 - 2*P2 - sum_d y2m_d*B_d + 2*sum_d ym_d*C_d ]

with per-core partials P1 = sum iv*y^2, P2 = sum iv*mu*y, C_d = sum_i iv*mu,
B_d = sum_i iv, S_d = sum_i y, T_d = sum_i y^2.  All partials are produced
on-device as fp32 accumulations; the host combine is O(128) work.

Precision: x, y, W, and hidden activations are fp16 (PE runs fp16 matmuls at
full rate; fp32 PSUM accumulation); mu, iv, and the product stage stay fp32.
Measured 6.4e-3 relative error on the final scalar vs the f32 reference
(the loss is a ~1e5x cancelling sum, so precision placement is load-bearing).
"""

import numpy as np

N_CORES = 8
N = 131072
D = 128
X_DIM = 128
H2 = 512
M = N // N_CORES          # rows per core = 16384
RG = 1024                 # rows per group
NG = M // RG              # groups per core = 16
NEG_SLOPE = 0.2

# Leaky-evacuation split knob: of every 10 (group, mlp, chunk) units,
# DVE_UNITS of them run on DVE (single fused custom op) instead of ACT.
DVE_UNITS = (0, 2, 4, 6)


def _dve_leaky(g, k, c):
    return ((g * 2 + k) * 4 + c) % 10 in DVE_UNITS


# engine for the P1/P2 product ops: gpsimd frees DVE for leaky evacuation
PROD_ON_POOL = False


def _peng(nc):
    return nc.gpsimd if PROD_ON_POOL else nc.vector


_leaky_op = None


def _get_leaky_op():
    """Custom DVE uop: out = max((in0 + s0) * imm2, in0 + s0) — fused
    bias-add + leaky-relu in one 1x pass straight from PSUM."""
    global _leaky_op
    if _leaky_op is not None:
        return _leaky_op
    import concourse.dve_ops as DO
    from concourse.dve_spec import C0, C2, Spec, Src0, maxx

    op = DO.DveOp(
        "LEAKY_BIAS_ANT",
        Spec(
            body=maxx((Src0 + C0) * C2, Src0 + C0),
            reference=lambda in0, in1, s0, s1, imm2: np.maximum(
                (in0.astype(np.float32) + s0) * imm2,
                in0.astype(np.float32) + s0),
        ),
        subdim=False,
        uops_sha={"v3": "28ce115f5da0f06f", "v4": ""},
    )
    DO.OPS.append(op)
    DO.CUSTOM_DVE_SPECS[op.name] = op.spec
    DO._SUB_OPCODE_FOR_NAME[op.name] = DO._CUSTOM_DVE_ROW_BASE + len(DO.OPS) - 1
    assert DO._SUB_OPCODE_FOR_NAME[op.name] < 0x20
    _leaky_op = op
    return op

_compiled = None


def _build():
    import concourse.bacc as bacc
    import concourse.tile as tile
    import concourse.mybir as mybir

    F32 = mybir.dt.float32
    F16 = mybir.dt.float16
    AF = mybir.ActivationFunctionType
    OP = mybir.AluOpType

    nc = bacc.Bacc("TRN2", target_bir_lowering=False, debug=False,
                   num_devices=N_CORES)

    x_d = nc.dram_tensor("x", [M, X_DIM], F32, kind="ExternalInput")
    y_d = nc.dram_tensor("y", [M, D], F32, kind="ExternalInput")
    w1_d = [nc.dram_tensor("W1m", [X_DIM, H2], F32, kind="ExternalInput"),
            nc.dram_tensor("W1v", [X_DIM, H2], F32, kind="ExternalInput")]
    b1_d = [nc.dram_tensor("b1m", [H2], F32, kind="ExternalInput"),
            nc.dram_tensor("b1v", [H2], F32, kind="ExternalInput")]
    w2_d = [nc.dram_tensor("W2m", [H2, D], F32, kind="ExternalInput"),
            nc.dram_tensor("W2v", [H2, D], F32, kind="ExternalInput")]
    b2_d = [nc.dram_tensor("b2m", [D], F32, kind="ExternalInput"),
            nc.dram_tensor("b2v", [D], F32, kind="ExternalInput")]
    out_d = nc.dram_tensor("out", [6, D, NG], F32, kind="ExternalOutput")

    with tile.TileContext(nc) as tc:
        with (
            tc.tile_pool(name="singles", bufs=1) as singles,
            tc.tile_pool(name="tposed", bufs=2) as tposed,
            tc.tile_pool(name="hidden", bufs=2) as hidden,
            tc.tile_pool(name="l2", bufs=2) as l2pool,
            tc.tile_pool(name="scratch", bufs=2) as scratch,
            tc.tile_pool(name="hpsum", bufs=2, space="PSUM") as hpsum,
            tc.tile_pool(name="l2psum", bufs=1, space="PSUM") as l2psum,
            tc.tile_pool(name="dram", bufs=1, space="DRAM") as dram,
        ):
            # fp16 row-major bounce buffers in DRAM (see loop below).
            xh_dram = dram.tile([M, X_DIM], F16, name="xh_dram")
            yh_dram = dram.tile([M, D], F16, name="yh_dram")

            def cast_and_transpose(g, xT, yT, nsub):
                # gpsimd (SWDGE) DMAs can cast, so one DRAM->DRAM casting
                # DMA per group produces the fp16 rows, and one big
                # DRAM->SBUF xbar transpose per group loads them as
                # [feature, row].  Per-instruction DGE overhead (~0.6-1us)
                # makes small transposes far more expensive than big ones.
                sub = RG // nsub
                for i in range(nsub):
                    rows = slice(g * RG + i * sub, g * RG + (i + 1) * sub)
                    nc.gpsimd.dma_start(xh_dram[rows, :], x_d[rows, :])
                    nc.gpsimd.dma_start(yh_dram[rows, :], y_d[rows, :])
                    nc.sync.dma_start_transpose(
                        xT[:, i * sub:(i + 1) * sub], xh_dram[rows, :])
                    nc.sync.dma_start_transpose(
                        yT[:, i * sub:(i + 1) * sub], yh_dram[rows, :])

            # Group 0's cast+transpose chain is emitted FIRST (quartered) so
            # the DMA engines deliver the first xT slab before the pile of
            # weight/bias loads — the first matmul only needs w1 and xT[q0].
            xT0 = tposed.tile([X_DIM, RG], F16, tag="xT", name="xT0")
            yT0 = tposed.tile([D, RG], F16, tag="yT", name="yT0")
            cast_and_transpose(0, xT0, yT0, nsub=4)

            # --- weights / biases: load f32, cast weights to fp16 ---
            w1h, w2h, b1t, b2t, nb2t = [], [], [], [], []
            for k in range(2):
                w1f = singles.tile([X_DIM, H2], F32, tag=f"w1f{k}")
                nc.scalar.dma_start(w1f[:], w1_d[k][:])
                w1 = singles.tile([X_DIM, H2], F16, tag=f"w1h{k}")
                nc.vector.tensor_copy(w1[:], w1f[:])
                w1h.append(w1)

                w2f = singles.tile([128, 4, D], F32, tag=f"w2f{k}")
                for c in range(4):
                    nc.scalar.dma_start(w2f[:, c, :], w2_d[k][c * 128:(c + 1) * 128, :])
                w2 = singles.tile([128, 4, D], F16, tag=f"w2h{k}")
                nc.vector.tensor_copy(w2[:], w2f[:])
                w2h.append(w2)

                bt = []
                for c in range(4):
                    b = singles.tile([128, 1], F32, tag=f"b1_{k}_{c}")
                    nc.scalar.dma_start(b[:], b1_d[k][c * 128:(c + 1) * 128].rearrange("(p one) -> p one", one=1))
                    bt.append(b)
                b1t.append(bt)

                b2 = singles.tile([D, 1], F32, tag=f"b2_{k}")
                nc.scalar.dma_start(b2[:], b2_d[k][:].rearrange("(p one) -> p one", one=1))
                b2t.append(b2)
                nb2 = singles.tile([D, 1], F32, tag=f"nb2_{k}")
                nc.vector.tensor_scalar(out=nb2[:], in0=b2[:], scalar1=-1.0,
                                        scalar2=None, op0=OP.mult)
                nb2t.append(nb2)

            # --- per-d partial accumulators, one column per group ---
            acc = {}
            for nm in ("P1", "P2", "C", "B", "S", "T"):
                acc_t = singles.tile([D, NG], F32, tag=f"acc_{nm}", name=f"acc_{nm}")
                acc[nm] = acc_t

            for g in range(NG):
                if g == 0:
                    xT, yT = xT0, yT0
                else:
                    xT = tposed.tile([X_DIM, RG], F16, tag="xT")
                    yT = tposed.tile([D, RG], F16, tag="yT")
                    cast_and_transpose(g, xT, yT, nsub=1)

                # --- layer 1 + leaky ---
                hT = []
                for k in range(2):
                    hk = []
                    for c in range(4):
                        hp = hpsum.tile([128, RG], F32, tag="hps")
                        for s in range(RG // 512):
                            nc.tensor.matmul(hp[:, s * 512:(s + 1) * 512],
                                             w1h[k][:, c * 128:(c + 1) * 128],
                                             xT[:, s * 512:(s + 1) * 512],
                                             start=True, stop=True)
                        ht = hidden.tile([128, RG], F16, tag=f"hT{k}{c}")
                        if _dve_leaky(g, k, c):
                            nc.vector._custom_dve(
                                _get_leaky_op(), out=ht[:], in0=hp[:],
                                s0=b1t[k][c][:], imm2=NEG_SLOPE)
                        else:
                            nc.scalar.activation(ht[:], hp[:], AF.Prelu,
                                                 bias=b1t[k][c][:], scale=1.0,
                                                 alpha=NEG_SLOPE)
                        hk.append(ht)
                    hT.append(hk)

                # --- layer 2 (accumulate over 4 chunks) ---
                mups = l2psum.tile([D, RG], F32, tag="mups")
                zps = l2psum.tile([D, RG], F32, tag="zps")
                for s in range(RG // 512):
                    for c in range(4):
                        nc.tensor.matmul(mups[:, s * 512:(s + 1) * 512],
                                         w2h[0][:, c, :],
                                         hT[0][c][:, s * 512:(s + 1) * 512],
                                         start=(c == 0), stop=(c == 3))
                for s in range(RG // 512):
                    for c in range(4):
                        nc.tensor.matmul(zps[:, s * 512:(s + 1) * 512],
                                         w2h[1][:, c, :],
                                         hT[1][c][:, s * 512:(s + 1) * 512],
                                         start=(c == 0), stop=(c == 3))

                # mu = psum + b2m (fp32); u = -tanh(psum + b2v); iv = exp(u)
                mu = l2pool.tile([D, RG], F32, tag="mu")
                nc.scalar.activation(mu[:], mups[:], AF.Identity, bias=b2t[0][:])
                u = l2pool.tile([D, RG], F32, tag="u")
                nc.scalar.activation(u[:], zps[:], AF.Tanh, bias=nb2t[1][:], scale=-1.0)
                iv = l2pool.tile([D, RG], F32, tag="iv")
                nc.scalar.activation(iv[:], u[:], AF.Exp,
                                     accum_out=acc["B"][:, g:g + 1])

                # --- product stage (fp32 internal, fp32 accumulators) ---
                q = scratch.tile([D, RG], F32, tag="q")
                nc.vector.scalar_tensor_tensor(
                    out=q[:], in0=iv[:], scalar=1.0, in1=mu[:],
                    op0=OP.mult, op1=OP.mult,
                    accum_out=acc["C"][:, g:g + 1])
                p2s = scratch.tile([D, RG], F32, tag="p2s")
                _peng(nc).scalar_tensor_tensor(
                    out=p2s[:], in0=q[:], scalar=1.0, in1=yT[:],
                    op0=OP.mult, op1=OP.mult,
                    accum_out=acc["P2"][:, g:g + 1])
                y2 = scratch.tile([D, RG], F16, tag="y2")
                nc.vector.scalar_tensor_tensor(
                    out=y2[:], in0=yT[:], scalar=1.0, in1=yT[:],
                    op0=OP.mult, op1=OP.mult)
                # T must sum the SAME fp16-rounded y2 tile P1 consumes:
                # fp16(y^2) rounding is biased, and in the combine the bias
                # only cancels against P1's if T carries it too.
                t2s = scratch.tile([D, RG], F16, tag="t2s")
                nc.vector.tensor_scalar(
                    out=t2s[:], in0=y2[:], scalar1=1.0, scalar2=None,
                    op0=OP.mult, op1=OP.add,
                    accum_out=acc["T"][:, g:g + 1])
                p1s = scratch.tile([D, RG], F32, tag="p1s")
                _peng(nc).scalar_tensor_tensor(
                    out=p1s[:], in0=iv[:], scalar=1.0, in1=y2[:],
                    op0=OP.mult, op1=OP.mult,
                    accum_out=acc["P1"][:, g:g + 1])
                ss = scratch.tile([D, RG], F16, tag="ss")
                # out = y*1; accum reduces out with op1 (add) along free dim
                nc.vector.tensor_scalar(
                    out=ss[:], in0=yT[:], scalar1=1.0, scalar2=None,
                    op0=OP.mult, op1=OP.add,
                    accum_out=acc["S"][:, g:g + 1])

            for i, nm in enumerate(("P1", "P2", "C", "B", "S", "T")):
                nc.sync.dma_start(out_d[i], acc[nm][:])

    nc.compile()
    return nc


def _get_compiled():
    global _compiled
    if _compiled is None:
        _compiled = _build()
    return _compiled


def kernel(x_samples, y_samples, W1m, b1m, W2m, b2m, W1v, b1v, W2v, b2v):
    from concourse.bass_utils import run_bass_kernel_spmd

    nc = _get_compiled()

    xs = np.ascontiguousarray(x_samples, dtype=np.float32)
    ys = np.ascontiguousarray(y_samples, dtype=np.float32)
    in_maps = []
    for i in range(N_CORES):
        sl = slice(i * M, (i + 1) * M)
        in_maps.append({
            "x": xs[sl], "y": ys[sl],
            "W1m": np.asarray(W1m, np.float32), "b1m": np.asarray(b1m, np.float32),
            "W2m": np.asarray(W2m, np.float32), "b2m": np.asarray(b2m, np.float32),
            "W1v": np.asarray(W1v, np.float32), "b1v": np.asarray(b1v, np.float32),
            "W2v": np.asarray(W2v, np.float32), "b2v": np.asarray(b2v, np.float32),
        })

    res = run_bass_kernel_spmd(nc, in_maps, list(range(N_CORES)))
    return combine([r["out"] for r in res.results])


def combine(outs):
    """Host-side gather: sum per-core [6, 128, NG] partials and finish the loss."""
    tot = np.sum([o.astype(np.float64) for o in outs], axis=(0, 3))
    P1, P2, C, B, S, T = tot
    ym = S / N
    y2m = T / N
    total = P1.sum() - 2.0 * P2.sum() - (y2m * B).sum() + 2.0 * (ym * C).sum()
    return np.float32(-0.5 * total / N)



# revision 2
# speedup vs baseline: 1.2233x; 1.2233x over previous
"""Trainium2 Bass kernel for nn_CLUB_816043786555 (CLUB loss).

Full-input contract: kernel(**inputs) takes the complete arrays, shards the
batch dim across 8 NeuronCores, runs a Bass/Tile kernel per core, and
combines tiny per-core partial sums on the host.

Math: with mu = leaky(x@W1m+b1m)@W2m+b2m, logvar = tanh(leaky(x@W1v+b1v)@W2v+b2v),
iv = exp(-logvar), ym_d = mean_i y, y2m_d = mean_i y^2:

  loss = -0.5/N * sum_{i,d} iv*(y^2 - 2*mu*y - y2m + 2*mu*ym)
       = -0.5/N * [ P1 - 2*P2 - sum_d y2m_d*B_d + 2*sum_d ym_d*C_d ]

with per-core partials P1 = sum iv*y^2, P2 = sum iv*mu*y, C_d = sum_i iv*mu,
B_d = sum_i iv, S_d = sum_i y, T_d = sum_i y^2.  All partials are produced
on-device as fp32 accumulations; the host combine is O(128) work.

Host-side prep (dtype staging only): x/y/W are cast to fp16 on the host —
identical rounding to the previous on-device cast path, but no DRAM bounce
traffic.  b2v is negated on host so tanh's bias slot can consume it.

Schedule: groups of RG=1024 rows flow through a software pipeline where
unit g runs L1(g) matmuls interleaved (per 128-wide hidden chunk) with
L2(g-1) matmuls so the PE never stalls (its p-state ramp resets on any gap).
L2's z-head (logvar) occupies interleave slots 0-1 and the mu-head slots
2-3, so tanh can free the z PSUM early while the fused q op
((mups+b2m)*iv via AFFINE_MUL_REDUCE) frees the mu PSUM one unit later.

Engine split per group (target ~100% of PE pace on ACT/DVE, Pool under):
  ACT : leaky x ~4.5 (Prelu, bias fused) + tanh + exp(->iv fp16, accum B)
  DVE : leaky x ~3.5 (custom op, bias fused) + q(affine from PSUM, accum C)
        + p2s (q*y, accum P2) + p1s (iv*y2, accum P1)   [2x fp16 modes]
  Pool: y2 (y*y), t2s (accum T), ss (accum S)           [SBUF-only]

Precision: fp16 everywhere except PSUM accumulation, mu (stays f32 inside
the fused affine), and the f32 partial accumulators; T sums the same
rounded fp16 y^2 tile that P1 consumes (bias cancellation).
"""

import numpy as np

N_CORES = 8
N = 131072
D = 128
X_DIM = 128
H2 = 512
M = N // N_CORES          # rows per core = 16384
RG = 1024                 # rows per group
NG = M // RG              # groups per core = 16
NEG_SLOPE = 0.2

# Per-group leaky->DVE unit assignment (units indexed u = c*2 + k in emission
# order).  Even groups run 4 units on DVE, odd groups 3 (avg 3.5/4.5 split).
LEAKY_DVE_EVEN = (1, 3, 5, 7)
LEAKY_DVE_ODD = (1, 3, 5)

# B = sum(iv) accumulated for free on the ACT exp op (sums pre-rounding f32
# exp values; P1/C consume the rounded fp16 iv — mismatch noise measured
# acceptable).  Set False to spend a Pool op on an exactly-consistent B.
USE_EXP_ACCUM_B = True


def _leaky_on_dve(g, u):
    return u in (LEAKY_DVE_EVEN if g % 2 == 0 else LEAKY_DVE_ODD)


_leaky_op = None


def _get_leaky_op():
    """Custom DVE uop: out = max((in0 + s0) * imm2, in0 + s0) — fused
    bias-add + leaky-relu in one 1x pass straight from PSUM."""
    global _leaky_op
    if _leaky_op is not None:
        return _leaky_op
    import concourse.dve_ops as DO
    from concourse.dve_spec import C0, C2, Spec, Src0, maxx

    op = DO.DveOp(
        "LEAKY_BIAS_ANT",
        Spec(
            body=maxx((Src0 + C0) * C2, Src0 + C0),
            reference=lambda in0, in1, s0, s1, imm2: np.maximum(
                (in0.astype(np.float32) + s0) * imm2,
                in0.astype(np.float32) + s0),
        ),
        subdim=False,
        uops_sha={"v3": "28ce115f5da0f06f", "v4": ""},
    )
    DO.OPS.append(op)
    DO.CUSTOM_DVE_SPECS[op.name] = op.spec
    DO._SUB_OPCODE_FOR_NAME[op.name] = DO._CUSTOM_DVE_ROW_BASE + len(DO.OPS) - 1
    assert DO._SUB_OPCODE_FOR_NAME[op.name] < 0x20
    _leaky_op = op
    return op


_compiled = None


def _build():
    import concourse.bacc as bacc
    import concourse.tile as tile
    import concourse.mybir as mybir

    F32 = mybir.dt.float32
    F16 = mybir.dt.float16
    AF = mybir.ActivationFunctionType
    OP = mybir.AluOpType

    nc = bacc.Bacc("TRN2", target_bir_lowering=False, debug=False,
                   num_devices=N_CORES)

    x_d = nc.dram_tensor("x16", [M, X_DIM], F16, kind="ExternalInput")
    y_d = nc.dram_tensor("y16", [M, D], F16, kind="ExternalInput")
    w1_d = [nc.dram_tensor("w1m16", [X_DIM, H2], F16, kind="ExternalInput"),
            nc.dram_tensor("w1v16", [X_DIM, H2], F16, kind="ExternalInput")]
    # W2 pre-shuffled on host to [128, 4*128]: w2[p, c*128+d] = W2[c*128+p, d]
    w2_d = [nc.dram_tensor("w2m16", [128, 4 * D], F16, kind="ExternalInput"),
            nc.dram_tensor("w2v16", [128, 4 * D], F16, kind="ExternalInput")]
    # b1 host-reshaped to [128, 4]: b1r[p, c] = b1[c*128+p]
    b1_d = [nc.dram_tensor("b1m_r", [128, 4], F32, kind="ExternalInput"),
            nc.dram_tensor("b1v_r", [128, 4], F32, kind="ExternalInput")]
    b2m_d = nc.dram_tensor("b2m_r", [D, 1], F32, kind="ExternalInput")
    nb2v_d = nc.dram_tensor("nb2v_r", [D, 1], F32, kind="ExternalInput")
    out_d = nc.dram_tensor("out", [6, D, NG], F32, kind="ExternalOutput")

    leaky_op = _get_leaky_op()

    with tile.TileContext(nc) as tc:
        with (
            tc.tile_pool(name="consts", bufs=1) as consts,
            tc.tile_pool(name="tposed", bufs=2) as tposed,
            tc.tile_pool(name="hidden", bufs=2) as hidden,
            tc.tile_pool(name="l2", bufs=2) as l2pool,
            tc.tile_pool(name="junk", bufs=1) as junk,
            tc.tile_pool(name="hpsum", bufs=2, space="PSUM") as hpsum,
            tc.tile_pool(name="l2psum", bufs=1, space="PSUM") as l2psum,
        ):
            def load_group(g):
                xT = tposed.tile([X_DIM, RG], F16, tag="xT")
                yT = tposed.tile([D, RG], F16, tag="yT")
                rows = slice(g * RG, (g + 1) * RG)
                nc.sync.dma_start_transpose(xT[:], x_d[rows, :])
                nc.sync.dma_start_transpose(yT[:], y_d[rows, :])
                return xT, yT

            cur = load_group(0)

            # --- weights / biases (already fp16/laid-out on host) ---
            w1h, w2h, b1t = [], [], []
            for k in range(2):
                w1 = consts.tile([X_DIM, H2], F16, tag=f"w1h{k}")
                nc.scalar.dma_start(w1[:], w1_d[k][:])
                w1h.append(w1)
                w2 = consts.tile([128, 4, D], F16, tag=f"w2h{k}")
                nc.scalar.dma_start(
                    w2[:], w2_d[k][:].rearrange("p (c d) -> p c d", c=4))
                w2h.append(w2)
                b1 = consts.tile([128, 4], F32, tag=f"b1_{k}")
                nc.scalar.dma_start(b1[:], b1_d[k][:])
                b1t.append(b1)
            b2m = consts.tile([D, 1], F32, tag="b2m")
            nc.scalar.dma_start(b2m[:], b2m_d[:])
            nb2v = consts.tile([D, 1], F32, tag="nb2v")
            nc.scalar.dma_start(nb2v[:], nb2v_d[:])

            acc = {}
            for nm in ("P1", "P2", "C", "B", "S", "T"):
                acc[nm] = consts.tile([D, NG], F32, tag=f"acc_{nm}",
                                      name=f"acc_{nm}")

            nxt = load_group(1)

            def emit_L1_chunk(g, c, xT, hts):
                for k in range(2):
                    hp = hpsum.tile([128, RG], F32, tag="hp")
                    for s in range(2):
                        nc.tensor.matmul(hp[:, s * 512:(s + 1) * 512],
                                         w1h[k][:, c * 128:(c + 1) * 128],
                                         xT[:, s * 512:(s + 1) * 512],
                                         start=True, stop=True)
                    ht = hidden.tile([128, RG], F16, tag=f"hT{k}{c}")
                    if _leaky_on_dve(g, c * 2 + k):
                        nc.vector._custom_dve(
                            leaky_op, out=ht[:], in0=hp[:],
                            s0=b1t[k][:, c:c + 1], imm2=NEG_SLOPE)
                    else:
                        nc.scalar.activation(ht[:], hp[:], AF.Prelu,
                                             bias=b1t[k][:, c:c + 1],
                                             scale=1.0, alpha=NEG_SLOPE)
                    hts[c * 2 + k] = ht

            def emit_L2_slot(slot, prev_hts, mups, zps):
                # slots 0,1 -> z-head (k=1), slots 2,3 -> mu-head (k=0)
                k = 1 if slot < 2 else 0
                ps = zps if k == 1 else mups
                for c in ((0, 1) if slot % 2 == 0 else (2, 3)):
                    for s in range(2):
                        nc.tensor.matmul(ps[:, s * 512:(s + 1) * 512],
                                         w2h[k][:, c, :],
                                         prev_hts[c * 2 + k][:, s * 512:(s + 1) * 512],
                                         start=(c == 0), stop=(c == 3))

            def emit_tanh_exp(g, zps):
                u = l2pool.tile([D, RG], F32, tag="u")
                nc.scalar.activation(u[:], zps[:], AF.Tanh,
                                     bias=nb2v[:], scale=-1.0)
                iv = l2pool.tile([D, RG], F16, tag="iv")
                if USE_EXP_ACCUM_B:
                    nc.scalar.activation(iv[:], u[:], AF.Exp,
                                         accum_out=acc["B"][:, g:g + 1])
                else:
                    nc.scalar.activation(iv[:], u[:], AF.Exp)
                return iv

            def emit_products(g, mups, iv, yT):
                # DVE: q = (mups + b2m) * iv  (mu never rounded), accum C
                q = l2pool.tile([D, RG], F16, tag="q")
                nc.vector.affine_mul_reduce(
                    out=q[:], accum_out=acc["C"][:, g:g + 1],
                    in0=mups[:], in1=iv[:], scale=1.0, bias=b2m[:])
                jA = junk.tile([D, RG], F16, tag="jA")
                nc.vector.scalar_tensor_tensor(
                    out=jA[:], in0=q[:], scalar=1.0, in1=yT[:],
                    op0=OP.mult, op1=OP.mult,
                    accum_out=acc["P2"][:, g:g + 1])
                # Pool: y2, then T/S accumulations (SBUF-only ops)
                y2 = l2pool.tile([D, RG], F16, tag="y2")
                nc.gpsimd.scalar_tensor_tensor(
                    out=y2[:], in0=yT[:], scalar=1.0, in1=yT[:],
                    op0=OP.mult, op1=OP.mult)
                jT = junk.tile([D, RG], F16, tag="jT")
                nc.gpsimd.tensor_scalar(
                    out=jT[:], in0=y2[:], scalar1=1.0, scalar2=None,
                    op0=OP.mult, op1=OP.add,
                    accum_out=acc["T"][:, g:g + 1])
                jS = junk.tile([D, RG], F16, tag="jS")
                nc.gpsimd.tensor_scalar(
                    out=jS[:], in0=yT[:], scalar1=1.0, scalar2=None,
                    op0=OP.mult, op1=OP.add,
                    accum_out=acc["S"][:, g:g + 1])
                if not USE_EXP_ACCUM_B:
                    jB = junk.tile([D, RG], F16, tag="jB")
                    nc.gpsimd.tensor_scalar(
                        out=jB[:], in0=iv[:], scalar1=1.0, scalar2=None,
                        op0=OP.mult, op1=OP.add,
                        accum_out=acc["B"][:, g:g + 1])
                # DVE: P1 = sum iv * y2 (consumes the rounded fp16 y2; T
                # sums the same tile so the rounding bias cancels)
                jP = junk.tile([D, RG], F16, tag="jP")
                nc.vector.scalar_tensor_tensor(
                    out=jP[:], in0=iv[:], scalar=1.0, in1=y2[:],
                    op0=OP.mult, op1=OP.mult,
                    accum_out=acc["P1"][:, g:g + 1])

            prev_hts = None
            prev_yT = None
            for g in range(NG):
                xT, yT = cur
                if prev_hts is not None:
                    mups = l2psum.tile([D, RG], F32, tag="mups")
                    zps = l2psum.tile([D, RG], F32, tag="zps")
                hts = {}
                for c in range(4):
                    emit_L1_chunk(g, c, xT, hts)
                    if prev_hts is not None:
                        emit_L2_slot(c, prev_hts, mups, zps)
                        if c == 1:
                            iv = emit_tanh_exp(g - 1, zps)
                if prev_hts is not None:
                    emit_products(g - 1, mups, iv, prev_yT)
                prev_hts = hts
                prev_yT = yT
                cur = nxt
                nxt = load_group(g + 2) if g + 2 < NG else None

            # drain: L2 + post-ops for the final group
            mups = l2psum.tile([D, RG], F32, tag="mups")
            zps = l2psum.tile([D, RG], F32, tag="zps")
            for slot in range(4):
                emit_L2_slot(slot, prev_hts, mups, zps)
                if slot == 1:
                    iv = emit_tanh_exp(NG - 1, zps)
            emit_products(NG - 1, mups, iv, prev_yT)

            for i, nm in enumerate(("P1", "P2", "C", "B", "S", "T")):
                nc.sync.dma_start(out_d[i], acc[nm][:])

    nc.compile()
    return nc


def _get_compiled():
    global _compiled
    if _compiled is None:
        _compiled = _build()
    return _compiled


def make_in_maps(x_samples, y_samples, W1m, b1m, W2m, b2m, W1v, b1v, W2v, b2v):
    """Host-side staging: shard x/y over cores, cast to fp16, lay out weights."""
    f16 = np.float16
    f32 = np.float32

    def w2_shuffle(W2):
        return np.ascontiguousarray(
            np.asarray(W2, f32).reshape(4, 128, D).transpose(1, 0, 2)
            .reshape(128, 4 * D).astype(f16))

    shared = {
        "w1m16": np.ascontiguousarray(np.asarray(W1m, f32).astype(f16)),
        "w1v16": np.ascontiguousarray(np.asarray(W1v, f32).astype(f16)),
        "w2m16": w2_shuffle(W2m),
        "w2v16": w2_shuffle(W2v),
        "b1m_r": np.ascontiguousarray(np.asarray(b1m, f32).reshape(4, 128).T),
        "b1v_r": np.ascontiguousarray(np.asarray(b1v, f32).reshape(4, 128).T),
        "b2m_r": np.ascontiguousarray(np.asarray(b2m, f32).reshape(D, 1)),
        "nb2v_r": np.ascontiguousarray(-np.asarray(b2v, f32).reshape(D, 1)),
    }
    xs = np.asarray(x_samples, f32).astype(f16)
    ys = np.asarray(y_samples, f32).astype(f16)
    in_maps = []
    for i in range(N_CORES):
        sl = slice(i * M, (i + 1) * M)
        m = {"x16": np.ascontiguousarray(xs[sl]),
             "y16": np.ascontiguousarray(ys[sl])}
        m.update(shared)
        in_maps.append(m)
    return in_maps


def kernel(x_samples, y_samples, W1m, b1m, W2m, b2m, W1v, b1v, W2v, b2v):
    from concourse.bass_utils import run_bass_kernel_spmd

    nc = _get_compiled()
    in_maps = make_in_maps(x_samples, y_samples, W1m, b1m, W2m, b2m,
                           W1v, b1v, W2v, b2v)
    res = run_bass_kernel_spmd(nc, in_maps, list(range(N_CORES)))
    return combine([r["out"] for r in res.results])


def combine(outs):
    """Host-side gather: sum per-core [6, 128, NG] partials and finish the loss."""
    tot = np.sum([o.astype(np.float64) for o in outs], axis=(0, 3))
    P1, P2, C, B, S, T = tot
    ym = S / N
    y2m = T / N
    total = P1.sum() - 2.0 * P2.sum() - (y2m * B).sum() + 2.0 * (ym * C).sum()
    return np.float32(-0.5 * total / N)
